# revision 23
# baseline (speedup 1.0000x reference)
"""Trainium2 Bass kernel for a context-LSTM decoder (fused single-loop design).

Model (B=256, T=256, I=H=1024, 4H=4096, F=512, NC=7):
    ctx   = v @ Wc.T + (bc + bi + bh)                      # [B, 4H], const over t
    per t: gates = x_t @ Wi.T + ctx + h @ Wh.T ; LSTM cell update
    out   = relu(h_T @ Wfa.T + bfa) @ Wfc.T + bfc          # [B, 7]

Strategy: pure data-parallel over batch, 32 rows per core, no collectives.

Key idea vs the v1 kernel: all three gate contributions accumulate in ONE
PSUM tile pair per step, and every matmul is column-tiled
(tile_position=(0,32j)) so four independent M=32 chunk-matmuls run
concurrently in the 128x128 PE array (~4x the PE efficiency of M=32 alone).
The x-projection for step t+LOOKAHEAD is emitted between the recurrence
matmuls of consecutive steps, so the PE never idles waiting for the cell-math
tail; there is no gx DRAM round trip at all.

PSUM layout per step: tile A rows = [i0|i1|f0|f1] (32 rows each), tile B =
[g0|g1|o0|o1], where e.g. i0 = i-gate columns 0:512. Activations then run as
[128,512]/[64,512] ops, and the cell products use same-base SBUF pairs plus
cross-quadrant DVE writes (verified legal+correct on HW by probe).
"""

import numpy as np
from contextlib import ExitStack

import concourse.bass as bass
import concourse.bacc as bacc
import concourse.mybir as mybir
import concourse.tile as tile
from concourse.bass_utils import run_bass_kernel_spmd

B, T, I = 256, 256, 1024
H = 1024
G = 4 * H
F = 512
NCLS = 7
NCORES = 8
BS = B // NCORES          # 32 batch rows per core
P = 128                   # partitions
KI = I // P               # 8 k-tiles over the input dim
KH = H // P               # 8 k-tiles over the hidden dim
TQ = 8                    # timesteps per x-prefetch DMA
LOOK = 1                  # steps of x-projection lookahead held in PSUM
F16 = mybir.dt.float16
F32 = mybir.dt.float32
AOP = mybir.AluOpType
AFT = mybir.ActivationFunctionType


def build_lstm(t_steps: int = T, rec_steps: int | None = None, reps: int = 1):
    assert rec_steps is None or rec_steps == t_steps
    assert t_steps % TQ == 0
    nc = bacc.Bacc("TRN2", target_bir_lowering=False, debug=False,
                   num_devices=NCORES)

    xT8 = nc.declare_dram_parameter("xT8", [t_steps // TQ, I, TQ, BS], F16,
                                    isOutput=False)
    vT = nc.declare_dram_parameter("vT", [I, BS], F16, isOutput=False)
    WiT = nc.declare_dram_parameter("WiT", [I, G], F16, isOutput=False)
    WhT = nc.declare_dram_parameter("WhT", [H, G], F16, isOutput=False)
    WcT = nc.declare_dram_parameter("WcT", [I, G], F16, isOutput=False)
    WfaT = nc.declare_dram_parameter("WfaT", [H, F], F16, isOutput=False)
    WfcT = nc.declare_dram_parameter("WfcT", [F, NCLS], F16, isOutput=False)
    bias = nc.declare_dram_parameter("bias_total", [G], F32, isOutput=False)
    bfa = nc.declare_dram_parameter("bfa", [F], F32, isOutput=False)
    bfc = nc.declare_dram_parameter("bfc", [NCLS], F32, isOutput=False)
    ident = nc.declare_dram_parameter("ident32", [BS, BS], F16, isOutput=False)
    out = nc.declare_dram_parameter("out", [BS, NCLS], F32, isOutput=True)

    # K-major views of DRAM tensors: i = k*128 + p
    xT8_r = xT8[:].rearrange("q (k p) s b -> q p k s b", p=P)
    WiT_r = WiT[:].rearrange("(k p) g -> p k g", p=P)
    WhT_r = WhT[:].rearrange("(k p) g -> p k g", p=P)
    WcT_r = WcT[:].rearrange("(k p) g -> p k g", p=P)
    WfaT_r = WfaT[:].rearrange("(k p) f -> p k f", p=P)
    WfcT_r = WfcT[:].rearrange("(q p) n -> p q n", p=P)
    vT_r = vT[:].rearrange("(k p) b -> p k b", p=P)

    def bcast(src_ap, rows):
        # read a [cols] DRAM vector into [rows, cols] SBUF (partition bcast)
        return bass.AP(tensor=src_ap.tensor, offset=src_ap.offset,
                       ap=[[0, rows]] + list(src_ap.ap))

    with tile.TileContext(nc) as tc, ExitStack() as ctx:
        consts = ctx.enter_context(tc.tile_pool(name="consts", bufs=1))

        # ---- resident constants ----
        bfa_rep = consts.tile([BS, F], F32, tag="bfa_rep")
        nc.sync.dma_start(out=bfa_rep, in_=bcast(bfa[:], BS))
        bfc_rep = consts.tile([BS, NCLS], F32, tag="bfc_rep")
        nc.sync.dma_start(out=bfc_rep, in_=bcast(bfc[:], BS))
        ident_sb = consts.tile([BS, BS], F16, tag="ident_sb")
        nc.sync.dma_start(out=ident_sb, in_=ident[:])
        identf_sb = consts.tile([BS, BS], F32, tag="identf_sb")
        nc.scalar.activation(out=identf_sb, in_=ident_sb, func=AFT.Copy)

        WiT_sb = consts.tile([P, KI, G], F16, tag="WiT_sb")
        for k in range(KI):
            nc.gpsimd.dma_start(out=WiT_sb[:, k, :], in_=WiT_r[:, k, :])
        WhT_sb = consts.tile([P, KH, G], F16, tag="WhT_sb")
        for k in range(KH):
            nc.gpsimd.dma_start(out=WhT_sb[:, k, :], in_=WhT_r[:, k, :])
        WfaT_sb = consts.tile([P, KH, F], F16, tag="WfaT_sb")
        nc.sync.dma_start(out=WfaT_sb, in_=WfaT_r)
        WfcT_sb = consts.tile([P, F // P, NCLS], F16, tag="WfcT_sb")
        nc.sync.dma_start(out=WfcT_sb, in_=WfcT_r)

        # ctx chunks: ctxc[c] = (v @ Wc.T + bias_total)[:, 512c:512c+512], fp16
        ctxc = [consts.tile([BS, 512], F16, tag=f"ctx{c}", name=f"ctx{c}")
                for c in range(8)]

        for _rep in range(reps):
            # ------- phase 0: ctx -------
            with nc.named_scope("phase0_ctx"):
                with (
                    tc.tile_pool(name="ph0_w", bufs=1) as ph0_w,
                    tc.tile_pool(name="ph0_misc", bufs=1) as ph0_misc,
                    tc.tile_pool(name="ph0_ps", bufs=2, space="PSUM") as ph0_ps,
                ):
                    v_sb = ph0_misc.tile([P, KI, BS], F16, tag="v_sb")
                    nc.sync.dma_start(out=v_sb, in_=vT_r)
                    bias_rep = ph0_misc.tile([BS, G], F32, tag="bias_rep")
                    nc.sync.dma_start(out=bias_rep, in_=bcast(bias[:], BS))
                    for half in range(2):
                        wc_t = ph0_w.tile([P, KI, 4, 512], F16, tag="wc")
                        for j in range(4):
                            c = 4 * half + j
                            nc.sync.dma_start(
                                out=wc_t[:, :, j, :],
                                in_=WcT_r[:, :, 512 * c:512 * (c + 1)])
                        ps0 = ph0_ps.tile([P, 512], F32, tag="ps0")
                        for k in range(KI):
                            for j in range(4):
                                nc.tensor.matmul(
                                    ps0[32 * j:32 * (j + 1), :],
                                    lhsT=v_sb[:, k, :],
                                    rhs=wc_t[:, k, j, :],
                                    start=(k == 0), stop=(k == KI - 1),
                                    tile_position=(0, 32 * j))
                        for j in range(4):
                            c = 4 * half + j
                            nc.vector.scalar_tensor_tensor(
                                out=ctxc[c], in0=ps0[32 * j:32 * (j + 1), :],
                                scalar=1.0,
                                in1=bias_rep[:, 512 * c:512 * (c + 1)],
                                op0=AOP.mult, op1=AOP.add)

            # ------- fused main loop -------
            with (
                tc.tile_pool(name="xq", bufs=3) as xqp,
                tc.tile_pool(name="state", bufs=1) as statep,
                tc.tile_pool(name="acts", bufs=3) as actp,
                tc.tile_pool(name="cell", bufs=3) as cellp,
                tc.tile_pool(name="hh", bufs=3) as hhp,
                tc.tile_pool(name="ht", bufs=4) as htp,
            ):
                c_st = statep.tile([P, 512], F16, tag="c_st")  # rows 64:128 used
                nc.vector.memset(c_st[64:128, :], 0.0)

                xq_tiles = {}

                def fetch_xq(q):
                    if q * TQ >= t_steps:
                        return
                    xt = xqp.tile([P, KI, TQ, BS], F16, tag="xq")
                    nc.gpsimd.dma_start(out=xt, in_=xT8_r[q])
                    xq_tiles[q] = xt

                ps_tiles = {}

                def x_part(t, close):
                    # gates(t) += ctx + x_t @ Wi.T  (start of PSUM accumulation)
                    psA = psp.tile([P, 512], F32, tag="psA", name="psA")
                    psB = psp.tile([P, 512], F32, tag="psB", name="psB")
                    ps_tiles[t] = (psA, psB)
                    xt = xq_tiles[t // TQ][:, :, t % TQ, :]
                    for ti, ps in ((0, psA), (1, psB)):
                        for j in range(4):
                            nc.tensor.matmul(
                                ps[32 * j:32 * (j + 1), :], lhsT=ident_sb,
                                rhs=ctxc[4 * ti + j], start=True, stop=False,
                                tile_position=(0, 32 * j))
                    for k in range(KI):
                        for ti, ps in ((0, psA), (1, psB)):
                            for j in range(4):
                                c = 4 * ti + j
                                nc.tensor.matmul(
                                    ps[32 * j:32 * (j + 1), :],
                                    lhsT=xt[:, k, :],
                                    rhs=WiT_sb[:, k, 512 * c:512 * (c + 1)],
                                    start=False,
                                    stop=(close and k == KI - 1),
                                    tile_position=(0, 32 * j))

                def h_part(t, hT):
                    # tile A completes first so sigmoid(i,f) and t1/t2 prep
                    # overlap tile B's matmuls; the post-matmul chain is then
                    # only tanh(g)/sigmoid(o) -> t2 -> c' -> tanh -> h.
                    psA, psB = ps_tiles[t]
                    for ti, ps in ((0, psA), (1, psB)):
                        for k in range(KH):
                            for j in range(4):
                                c = 4 * ti + j
                                nc.tensor.matmul(
                                    ps[32 * j:32 * (j + 1), :],
                                    lhsT=hT[k],
                                    rhs=WhT_sb[:, k, 512 * c:512 * (c + 1)],
                                    start=False, stop=(k == KH - 1),
                                    tile_position=(0, 32 * j))

                def cell(t):
                    # psA = [i0|i1|f0|f1], psB = [g0|g1|o0|o1]; tile B's gates
                    # are ready ~1.8us before tile A's, so tanh(g)/sigmoid(o)
                    # run under tile A's matmuls.
                    psA, psB = ps_tiles.pop(t)
                    sA = actp.tile([P, 512], F16, tag="sA", name="sA")
                    nc.scalar.activation(out=sA, in_=psA, func=AFT.Sigmoid)
                    # t1 = sigmoid(f) * c  (only needs psA: runs under tile B)
                    t1t = cellp.tile([P, 512], F16, tag="t1t", name="t1t")
                    nc.vector.tensor_tensor(out=t1t[64:128, :],
                                            in0=sA[64:128, :],
                                            in1=c_st[64:128, :], op=AOP.mult)
                    gb = actp.tile([P, 512], F16, tag="gb", name="gb")
                    nc.scalar.activation(out=gb[0:64, :], in_=psB[0:64, :],
                                         func=AFT.Tanh)
                    nc.scalar.activation(out=gb[64:128, :], in_=psB[64:128, :],
                                         func=AFT.Sigmoid)
                    # t2 = sigmoid(i) * tanh(g)  -> rows 64:128  (fp16 chain)
                    t2t = cellp.tile([P, 512], F16, tag="t2t", name="t2t")
                    nc.vector.tensor_tensor(out=t2t[64:128, :],
                                            in0=sA[0:64, :], in1=gb[0:64, :],
                                            op=AOP.mult)
                    nc.vector.tensor_tensor(out=c_st[64:128, :],
                                            in0=t1t[64:128, :],
                                            in1=t2t[64:128, :], op=AOP.add)
                    tct = cellp.tile([P, 512], F16, tag="tct", name="tct")
                    nc.scalar.activation(out=tct[64:128, :],
                                         in_=c_st[64:128, :], func=AFT.Tanh)
                    # h halves (fp32: the PE transpose path is fp32-only)
                    h0 = hhp.tile([BS, 512], F32, tag="h0", name="h0")
                    nc.vector.tensor_tensor(out=h0, in0=gb[64:96, :],
                                            in1=tct[64:96, :], op=AOP.mult)
                    h1 = hhp.tile([BS, 512], F32, tag="h1", name="h1")
                    nc.vector.tensor_tensor(out=h1, in0=gb[96:128, :],
                                            in1=tct[96:128, :], op=AOP.mult)
                    return h0, h1

                def pe_transpose(h0, h1):
                    # transpose h on the (otherwise stalled) PE: 8
                    # is_transpose matmuls into an fp32 PSUM tile, then two
                    # ACT copies to SBUF fp16. Emitted AFTER the next x_part
                    # so the PE stream stays busy while hh is computed.
                    psT = psp.tile([P, KH, BS], F32, tag="psT", name="psT",
                                   bufs=2)
                    for k in range(KH):
                        hsrc = (h0 if k < 4 else h1)[:, 128 * (k % 4):
                                                     128 * (k % 4) + 128]
                        nc.tensor.transpose(out=psT[:, k, :], in_=hsrc,
                                            identity=identf_sb)
                    htsb = htp.tile([P, KH, BS], F16, tag="ht", name="htsb")
                    nc.scalar.activation(out=htsb[:, 0:4, :],
                                         in_=psT[:, 0:4, :], func=AFT.Copy)
                    nc.scalar.activation(out=htsb[:, 4:8, :],
                                         in_=psT[:, 4:8, :], func=AFT.Copy)
                    return [htsb[:, k, :] for k in range(KH)]

                # prologue: x prefetch + LOOK steps of x-projection
                with tc.tile_pool(name="ps", bufs=3, space="PSUM") as psp:
                    fetch_xq(0)
                    fetch_xq(1)
                    for t in range(LOOK):
                        x_part(t, close=(t == 0))

                    pend = None
                    for t in range(t_steps):
                        tp = t + LOOK
                        if tp < t_steps:
                            if tp % TQ == 0:
                                fetch_xq(tp // TQ + 1)
                            x_part(tp, close=False)
                        if pend is not None:
                            hT = pe_transpose(*pend)
                        if t > 0:
                            h_part(t, hT)
                        pend = cell(t)
                    hT = pe_transpose(*pend)

                # ------- head -------
                with nc.named_scope("head"):
                    with tc.tile_pool(name="head_ps", bufs=1,
                                      space="PSUM") as hps:
                        ps_f = hps.tile([BS, F], F32, tag="ps_f")
                        for k in range(KH):
                            nc.tensor.matmul(ps_f, lhsT=hT[k],
                                             rhs=WfaT_sb[:, k, :],
                                             start=(k == 0),
                                             stop=(k == KH - 1))
                        x1 = cellp.tile([BS, F], F32, tag="x1", bufs=1)
                        nc.vector.scalar_tensor_tensor(
                            out=x1, in0=ps_f, scalar=1.0, in1=bfa_rep,
                            op0=AOP.mult, op1=AOP.add)
                        x1r = cellp.tile([BS, F], F16, tag="x1r", bufs=1)
                        nc.scalar.activation(out=x1r, in_=x1, func=AFT.Relu)
                        x1T = htp.tile([P, F // P, BS], F16, tag="x1T",
                                       bufs=1)
                        nc.sync.dma_start_transpose(out=x1T, in_=x1r)
                        ps_o = hps.tile([BS, NCLS], F32, tag="ps_o")
                        for q in range(F // P):
                            nc.tensor.matmul(ps_o, lhsT=x1T[:, q, :],
                                             rhs=WfcT_sb[:, q, :],
                                             start=(q == 0),
                                             stop=(q == F // P - 1))
                        out_sb = cellp.tile([BS, NCLS], F32, tag="out_sb",
                                            bufs=1)
                        nc.vector.scalar_tensor_tensor(
                            out=out_sb, in0=ps_o, scalar=1.0, in1=bfc_rep,
                            op0=AOP.mult, op1=AOP.add)
                        nc.sync.dma_start(out=out[:], in_=out_sb)

    nc.compile()
    return nc


def make_in_maps(inputs: dict, t_steps: int = T):
    """Shard + lay out the full inputs for the 8 cores (host-side numpy)."""
    x = np.asarray(inputs["i_features"], np.float32)[:, :t_steps, :]
    v = np.asarray(inputs["v_features"], np.float32)
    Wi, bi = np.asarray(inputs["Wi"], np.float32), np.asarray(inputs["bi"], np.float32)
    Wh, bh = np.asarray(inputs["Wh"], np.float32), np.asarray(inputs["bh"], np.float32)
    Wc, bc = np.asarray(inputs["Wc"], np.float32), np.asarray(inputs["bc"], np.float32)
    Wfa, bfa = np.asarray(inputs["Wfa"], np.float32), np.asarray(inputs["bfa"], np.float32)
    Wfc, bfc = np.asarray(inputs["Wfc"], np.float32), np.asarray(inputs["bfc"], np.float32)

    shared = {
        "WiT": np.ascontiguousarray(Wi.T).astype(np.float16),
        "WhT": np.ascontiguousarray(Wh.T).astype(np.float16),
        "WcT": np.ascontiguousarray(Wc.T).astype(np.float16),
        "WfaT": np.ascontiguousarray(Wfa.T).astype(np.float16),
        "WfcT": np.ascontiguousarray(Wfc.T).astype(np.float16),
        "bias_total": (bi + bh + bc).astype(np.float32),
        "bfa": bfa.astype(np.float32),
        "bfc": bfc.astype(np.float32),
        "ident32": np.eye(BS, dtype=np.float16),
    }
    in_maps = []
    nb = x.shape[0] // BS
    for s in range(nb):
        xs = x[s * BS:(s + 1) * BS]                      # [BS, t, I]
        xT = np.ascontiguousarray(xs.transpose(1, 2, 0))  # [t, I, BS]
        xT8 = np.ascontiguousarray(
            xT.reshape(t_steps // TQ, TQ, I, BS).transpose(0, 2, 1, 3))
        in_maps.append({
            "xT8": xT8.astype(np.float16),
            "vT": np.ascontiguousarray(v[s * BS:(s + 1) * BS].T).astype(np.float16),
            **shared,
        })
    return in_maps


_NC_CACHE = {}


def finish_output(per_core_outs: list) -> np.ndarray:
    """Host-side gather of per-core 'out' tensors into the full [B, NC] result."""
    return np.concatenate(per_core_outs, axis=0).astype(np.float32)


def kernel(**inputs) -> np.ndarray:
    in_maps = make_in_maps(inputs, T)
    if T not in _NC_CACHE:
        _NC_CACHE[T] = build_lstm(T)
    nc = _NC_CACHE[T]
    res = run_bass_kernel_spmd(nc, in_maps, core_ids=list(range(NCORES)))
    return finish_output([r["out"] for r in res.results])


# revision 24
# speedup vs baseline: 1.0222x; 1.0222x over previous
"""Trainium2 Bass kernel for a context-LSTM decoder (fused single-loop design).

Model (B=256, T=256, I=H=1024, 4H=4096, F=512, NC=7):
    ctx   = v @ Wc.T + (bc + bi + bh)                      # [B, 4H], const over t
    per t: gates = x_t @ Wi.T + ctx + h @ Wh.T ; LSTM cell update
    out   = relu(h_T @ Wfa.T + bfa) @ Wfc.T + bfc          # [B, 7]

Strategy: pure data-parallel over batch, 32 rows per core, no collectives.

Key idea vs the v1 kernel: all three gate contributions accumulate in ONE
PSUM tile pair per step, and every matmul is column-tiled
(tile_position=(0,32j)) so four independent M=32 chunk-matmuls run
concurrently in the 128x128 PE array (~4x the PE efficiency of M=32 alone).
The x-projection for step t+LOOKAHEAD is emitted between the recurrence
matmuls of consecutive steps, so the PE never idles waiting for the cell-math
tail; there is no gx DRAM round trip at all.

PSUM layout per step: tile A rows = [i0|i1|f0|f1] (32 rows each), tile B =
[g0|g1|o0|o1], where e.g. i0 = i-gate columns 0:512. Activations then run as
[128,512]/[64,512] ops, and the cell products use same-base SBUF pairs plus
cross-quadrant DVE writes (verified legal+correct on HW by probe).
"""

import numpy as np
from contextlib import ExitStack

import concourse.bass as bass
import concourse.bacc as bacc
import concourse.mybir as mybir
import concourse.tile as tile
from concourse.bass_utils import run_bass_kernel_spmd

B, T, I = 256, 256, 1024
H = 1024
G = 4 * H
F = 512
NCLS = 7
NCORES = 8
BS = B // NCORES          # 32 batch rows per core
P = 128                   # partitions
KI = I // P               # 8 k-tiles over the input dim
KH = H // P               # 8 k-tiles over the hidden dim
TQ = 8                    # timesteps per x-prefetch DMA
LOOK = 1                  # steps of x-projection lookahead held in PSUM
F16 = mybir.dt.float16
F32 = mybir.dt.float32
AOP = mybir.AluOpType
AFT = mybir.ActivationFunctionType


def build_lstm(t_steps: int = T, rec_steps: int | None = None, reps: int = 1):
    assert rec_steps is None or rec_steps == t_steps
    assert t_steps % TQ == 0
    nc = bacc.Bacc("TRN2", target_bir_lowering=False, debug=False,
                   num_devices=NCORES)

    xT8 = nc.declare_dram_parameter("xT8", [t_steps // TQ, I, TQ, BS], F16,
                                    isOutput=False)
    vT = nc.declare_dram_parameter("vT", [I, BS], F16, isOutput=False)
    WiT = nc.declare_dram_parameter("WiT", [I, G], F16, isOutput=False)
    WhT = nc.declare_dram_parameter("WhT", [H, G], F16, isOutput=False)
    WcT = nc.declare_dram_parameter("WcT", [I, G], F16, isOutput=False)
    WfaT = nc.declare_dram_parameter("WfaT", [H, F], F16, isOutput=False)
    WfcT = nc.declare_dram_parameter("WfcT", [F, NCLS], F16, isOutput=False)
    bias = nc.declare_dram_parameter("bias_total", [G], F32, isOutput=False)
    bfa = nc.declare_dram_parameter("bfa", [F], F32, isOutput=False)
    bfc = nc.declare_dram_parameter("bfc", [NCLS], F32, isOutput=False)
    ident = nc.declare_dram_parameter("ident32", [BS, BS], F16, isOutput=False)
    out = nc.declare_dram_parameter("out", [BS, NCLS], F32, isOutput=True)

    # K-major views of DRAM tensors: i = k*128 + p
    xT8_r = xT8[:].rearrange("q (k p) s b -> q p k s b", p=P)
    WiT_r = WiT[:].rearrange("(k p) g -> p k g", p=P)
    WhT_r = WhT[:].rearrange("(k p) g -> p k g", p=P)
    WcT_r = WcT[:].rearrange("(k p) g -> p k g", p=P)
    WfaT_r = WfaT[:].rearrange("(k p) f -> p k f", p=P)
    WfcT_r = WfcT[:].rearrange("(q p) n -> p q n", p=P)
    vT_r = vT[:].rearrange("(k p) b -> p k b", p=P)

    def bcast(src_ap, rows):
        # read a [cols] DRAM vector into [rows, cols] SBUF (partition bcast)
        return bass.AP(tensor=src_ap.tensor, offset=src_ap.offset,
                       ap=[[0, rows]] + list(src_ap.ap))

    with tile.TileContext(nc) as tc, ExitStack() as ctx:
        consts = ctx.enter_context(tc.tile_pool(name="consts", bufs=1))

        # ---- resident constants ----
        bfa_rep = consts.tile([BS, F], F32, tag="bfa_rep")
        nc.sync.dma_start(out=bfa_rep, in_=bcast(bfa[:], BS))
        bfc_rep = consts.tile([BS, NCLS], F32, tag="bfc_rep")
        nc.sync.dma_start(out=bfc_rep, in_=bcast(bfc[:], BS))
        ident_sb = consts.tile([BS, BS], F16, tag="ident_sb")
        nc.sync.dma_start(out=ident_sb, in_=ident[:])
        identf_sb = consts.tile([BS, BS], F32, tag="identf_sb")
        nc.scalar.activation(out=identf_sb, in_=ident_sb, func=AFT.Copy)

        WiT_sb = consts.tile([P, KI, G], F16, tag="WiT_sb")
        for k in range(KI):
            nc.sync.dma_start(out=WiT_sb[:, k, :], in_=WiT_r[:, k, :])
        WhT_sb = consts.tile([P, KH, G], F16, tag="WhT_sb")
        for k in range(KH):
            nc.sync.dma_start(out=WhT_sb[:, k, :], in_=WhT_r[:, k, :])
        WfaT_sb = consts.tile([P, KH, F], F16, tag="WfaT_sb")
        nc.sync.dma_start(out=WfaT_sb, in_=WfaT_r)
        WfcT_sb = consts.tile([P, F // P, NCLS], F16, tag="WfcT_sb")
        nc.sync.dma_start(out=WfcT_sb, in_=WfcT_r)

        # ctx chunks: ctxc[c] = (v @ Wc.T + bias_total)[:, 512c:512c+512], fp16
        ctxc = [consts.tile([BS, 512], F16, tag=f"ctx{c}", name=f"ctx{c}")
                for c in range(8)]

        for _rep in range(reps):
            # ------- phase 0: ctx -------
            with nc.named_scope("phase0_ctx"):
                with (
                    tc.tile_pool(name="ph0_w", bufs=2) as ph0_w,
                    tc.tile_pool(name="ph0_misc", bufs=1) as ph0_misc,
                    tc.tile_pool(name="ph0_ps", bufs=2, space="PSUM") as ph0_ps,
                ):
                    v_sb = ph0_misc.tile([P, KI, BS], F16, tag="v_sb")
                    nc.sync.dma_start(out=v_sb, in_=vT_r)
                    bias_rep = ph0_misc.tile([BS, G], F32, tag="bias_rep")
                    nc.sync.dma_start(out=bias_rep, in_=bcast(bias[:], BS))
                    for c in range(8):
                        wc_t = ph0_w.tile([P, KI, 512], F16, tag="wc")
                        nc.sync.dma_start(
                            out=wc_t, in_=WcT_r[:, :, 512 * c:512 * (c + 1)])
                        ps0 = ph0_ps.tile([BS, 512], F32, tag="ps0")
                        for k in range(KI):
                            nc.tensor.matmul(ps0, lhsT=v_sb[:, k, :],
                                             rhs=wc_t[:, k, :],
                                             start=(k == 0),
                                             stop=(k == KI - 1))
                        nc.vector.scalar_tensor_tensor(
                            out=ctxc[c], in0=ps0, scalar=1.0,
                            in1=bias_rep[:, 512 * c:512 * (c + 1)],
                            op0=AOP.mult, op1=AOP.add)

            # ------- fused main loop -------
            with (
                tc.tile_pool(name="xq", bufs=3) as xqp,
                tc.tile_pool(name="state", bufs=1) as statep,
                tc.tile_pool(name="acts", bufs=3) as actp,
                tc.tile_pool(name="cell", bufs=3) as cellp,
                tc.tile_pool(name="hh", bufs=3) as hhp,
                tc.tile_pool(name="ht", bufs=4) as htp,
            ):
                c_st = statep.tile([P, 512], F16, tag="c_st")  # rows 64:128 used
                nc.vector.memset(c_st[64:128, :], 0.0)

                xq_tiles = {}

                def fetch_xq(q):
                    if q * TQ >= t_steps:
                        return
                    xt = xqp.tile([P, KI, TQ, BS], F16, tag="xq")
                    nc.gpsimd.dma_start(out=xt, in_=xT8_r[q])
                    xq_tiles[q] = xt

                ps_tiles = {}

                def x_part(t, close):
                    # gates(t) += ctx + x_t @ Wi.T  (start of PSUM accumulation)
                    psA = psp.tile([P, 512], F32, tag="psA", name="psA")
                    psB = psp.tile([P, 512], F32, tag="psB", name="psB")
                    ps_tiles[t] = (psA, psB)
                    xt = xq_tiles[t // TQ][:, :, t % TQ, :]
                    for ti, ps in ((0, psA), (1, psB)):
                        for j in range(4):
                            nc.tensor.matmul(
                                ps[32 * j:32 * (j + 1), :], lhsT=ident_sb,
                                rhs=ctxc[4 * ti + j], start=True, stop=False,
                                tile_position=(0, 32 * j))
                    for k in range(KI):
                        for ti, ps in ((0, psA), (1, psB)):
                            for j in range(4):
                                c = 4 * ti + j
                                nc.tensor.matmul(
                                    ps[32 * j:32 * (j + 1), :],
                                    lhsT=xt[:, k, :],
                                    rhs=WiT_sb[:, k, 512 * c:512 * (c + 1)],
                                    start=False,
                                    stop=(close and k == KI - 1),
                                    tile_position=(0, 32 * j))

                def h_part(t, hT):
                    # tile A completes first so sigmoid(i,f) and t1/t2 prep
                    # overlap tile B's matmuls; the post-matmul chain is then
                    # only tanh(g)/sigmoid(o) -> t2 -> c' -> tanh -> h.
                    psA, psB = ps_tiles[t]
                    for ti, ps in ((0, psA), (1, psB)):
                        for k in range(KH):
                            for j in range(4):
                                c = 4 * ti + j
                                nc.tensor.matmul(
                                    ps[32 * j:32 * (j + 1), :],
                                    lhsT=hT[k],
                                    rhs=WhT_sb[:, k, 512 * c:512 * (c + 1)],
                                    start=False, stop=(k == KH - 1),
                                    tile_position=(0, 32 * j))

                def cell(t):
                    # psA = [i0|i1|f0|f1], psB = [g0|g1|o0|o1]; tile B's gates
                    # are ready ~1.8us before tile A's, so tanh(g)/sigmoid(o)
                    # run under tile A's matmuls.
                    psA, psB = ps_tiles.pop(t)
                    sA = actp.tile([P, 512], F16, tag="sA", name="sA")
                    nc.scalar.activation(out=sA, in_=psA, func=AFT.Sigmoid)
                    # t1 = sigmoid(f) * c  (only needs psA: runs under tile B)
                    t1t = cellp.tile([P, 512], F16, tag="t1t", name="t1t")
                    nc.vector.tensor_tensor(out=t1t[64:128, :],
                                            in0=sA[64:128, :],
                                            in1=c_st[64:128, :], op=AOP.mult)
                    gb = actp.tile([P, 512], F16, tag="gb", name="gb")
                    nc.scalar.activation(out=gb[0:64, :], in_=psB[0:64, :],
                                         func=AFT.Tanh)
                    nc.scalar.activation(out=gb[64:128, :], in_=psB[64:128, :],
                                         func=AFT.Sigmoid)
                    # t2 = sigmoid(i) * tanh(g)  -> rows 64:128  (fp16 chain)
                    t2t = cellp.tile([P, 512], F16, tag="t2t", name="t2t")
                    nc.vector.tensor_tensor(out=t2t[64:128, :],
                                            in0=sA[0:64, :], in1=gb[0:64, :],
                                            op=AOP.mult)
                    nc.vector.tensor_tensor(out=c_st[64:128, :],
                                            in0=t1t[64:128, :],
                                            in1=t2t[64:128, :], op=AOP.add)
                    tct = cellp.tile([P, 512], F16, tag="tct", name="tct")
                    nc.scalar.activation(out=tct[64:128, :],
                                         in_=c_st[64:128, :], func=AFT.Tanh)
                    # h halves (fp32: the PE transpose path is fp32-only),
                    # split into 256-col pieces so the first PE transposes
                    # (k=0,1) can start before the rest of h is computed.
                    h0 = hhp.tile([BS, 512], F32, tag="h0", name="h0")
                    h1 = hhp.tile([BS, 512], F32, tag="h1", name="h1")
                    for q in range(2):
                        cs = slice(256 * q, 256 * (q + 1))
                        nc.vector.tensor_tensor(out=h0[:, cs],
                                                in0=gb[64:96, cs],
                                                in1=tct[64:96, cs],
                                                op=AOP.mult)
                    for q in range(2):
                        cs = slice(256 * q, 256 * (q + 1))
                        nc.vector.tensor_tensor(out=h1[:, cs],
                                                in0=gb[96:128, cs],
                                                in1=tct[96:128, cs],
                                                op=AOP.mult)
                    return h0, h1

                def pe_transpose(h0, h1):
                    # transpose h on the (otherwise stalled) PE: 8
                    # is_transpose matmuls into an fp32 PSUM tile, then two
                    # ACT copies to SBUF fp16. Emitted AFTER the next x_part
                    # so the PE stream stays busy while hh is computed.
                    psT = psp.tile([P, KH, BS], F32, tag="psT", name="psT",
                                   bufs=2)
                    for k in range(KH):
                        hsrc = (h0 if k < 4 else h1)[:, 128 * (k % 4):
                                                     128 * (k % 4) + 128]
                        nc.tensor.transpose(out=psT[:, k, :], in_=hsrc,
                                            identity=identf_sb)
                    htsb = htp.tile([P, KH, BS], F16, tag="ht", name="htsb")
                    for q in range(4):
                        nc.scalar.activation(out=htsb[:, 2 * q:2 * q + 2, :],
                                             in_=psT[:, 2 * q:2 * q + 2, :],
                                             func=AFT.Copy)
                    return [htsb[:, k, :] for k in range(KH)]

                # prologue: x prefetch + LOOK steps of x-projection
                with tc.tile_pool(name="ps", bufs=3, space="PSUM") as psp:
                    fetch_xq(0)
                    fetch_xq(1)
                    for t in range(LOOK):
                        x_part(t, close=(t == 0))

                    pend = None
                    for t in range(t_steps):
                        tp = t + LOOK
                        if tp < t_steps:
                            if tp % TQ == 0:
                                fetch_xq(tp // TQ + 1)
                            x_part(tp, close=False)
                        if pend is not None:
                            hT = pe_transpose(*pend)
                        if t > 0:
                            h_part(t, hT)
                        pend = cell(t)
                    hT = pe_transpose(*pend)

                # ------- head -------
                with nc.named_scope("head"):
                    with tc.tile_pool(name="head_ps", bufs=1,
                                      space="PSUM") as hps:
                        ps_f = hps.tile([BS, F], F32, tag="ps_f")
                        for k in range(KH):
                            nc.tensor.matmul(ps_f, lhsT=hT[k],
                                             rhs=WfaT_sb[:, k, :],
                                             start=(k == 0),
                                             stop=(k == KH - 1))
                        x1 = cellp.tile([BS, F], F32, tag="x1", bufs=1)
                        nc.vector.scalar_tensor_tensor(
                            out=x1, in0=ps_f, scalar=1.0, in1=bfa_rep,
                            op0=AOP.mult, op1=AOP.add)
                        x1r = cellp.tile([BS, F], F16, tag="x1r", bufs=1)
                        nc.scalar.activation(out=x1r, in_=x1, func=AFT.Relu)
                        x1T = htp.tile([P, F // P, BS], F16, tag="x1T",
                                       bufs=1)
                        nc.sync.dma_start_transpose(out=x1T, in_=x1r)
                        ps_o = hps.tile([BS, NCLS], F32, tag="ps_o")
                        for q in range(F // P):
                            nc.tensor.matmul(ps_o, lhsT=x1T[:, q, :],
                                             rhs=WfcT_sb[:, q, :],
                                             start=(q == 0),
                                             stop=(q == F // P - 1))
                        out_sb = cellp.tile([BS, NCLS], F32, tag="out_sb",
                                            bufs=1)
                        nc.vector.scalar_tensor_tensor(
                            out=out_sb, in0=ps_o, scalar=1.0, in1=bfc_rep,
                            op0=AOP.mult, op1=AOP.add)
                        nc.sync.dma_start(out=out[:], in_=out_sb)

    nc.compile()
    return nc


def make_in_maps(inputs: dict, t_steps: int = T):
    """Shard + lay out the full inputs for the 8 cores (host-side numpy)."""
    x = np.asarray(inputs["i_features"], np.float32)[:, :t_steps, :]
    v = np.asarray(inputs["v_features"], np.float32)
    Wi, bi = np.asarray(inputs["Wi"], np.float32), np.asarray(inputs["bi"], np.float32)
    Wh, bh = np.asarray(inputs["Wh"], np.float32), np.asarray(inputs["bh"], np.float32)
    Wc, bc = np.asarray(inputs["Wc"], np.float32), np.asarray(inputs["bc"], np.float32)
    Wfa, bfa = np.asarray(inputs["Wfa"], np.float32), np.asarray(inputs["bfa"], np.float32)
    Wfc, bfc = np.asarray(inputs["Wfc"], np.float32), np.asarray(inputs["bfc"], np.float32)

    shared = {
        "WiT": np.ascontiguousarray(Wi.T).astype(np.float16),
        "WhT": np.ascontiguousarray(Wh.T).astype(np.float16),
        "WcT": np.ascontiguousarray(Wc.T).astype(np.float16),
        "WfaT": np.ascontiguousarray(Wfa.T).astype(np.float16),
        "WfcT": np.ascontiguousarray(Wfc.T).astype(np.float16),
        "bias_total": (bi + bh + bc).astype(np.float32),
        "bfa": bfa.astype(np.float32),
        "bfc": bfc.astype(np.float32),
        "ident32": np.eye(BS, dtype=np.float16),
    }
    in_maps = []
    nb = x.shape[0] // BS
    for s in range(nb):
        xs = x[s * BS:(s + 1) * BS]                      # [BS, t, I]
        xT = np.ascontiguousarray(xs.transpose(1, 2, 0))  # [t, I, BS]
        xT8 = np.ascontiguousarray(
            xT.reshape(t_steps // TQ, TQ, I, BS).transpose(0, 2, 1, 3))
        in_maps.append({
            "xT8": xT8.astype(np.float16),
            "vT": np.ascontiguousarray(v[s * BS:(s + 1) * BS].T).astype(np.float16),
            **shared,
        })
    return in_maps


_NC_CACHE = {}


def finish_output(per_core_outs: list) -> np.ndarray:
    """Host-side gather of per-core 'out' tensors into the full [B, NC] result."""
    return np.concatenate(per_core_outs, axis=0).astype(np.float32)


def kernel(**inputs) -> np.ndarray:
    in_maps = make_in_maps(inputs, T)
    if T not in _NC_CACHE:
        _NC_CACHE[T] = build_lstm(T)
    nc = _NC_CACHE[T]
    res = run_bass_kernel_spmd(nc, in_maps, core_ids=list(range(NCORES)))
    return finish_output([r["out"] for r in res.results])


# revision 25
# speedup vs baseline: 1.0681x; 1.0449x over previous
"""Trainium2 Bass kernel for a context-LSTM decoder (fused single-loop design).

Model (B=256, T=256, I=H=1024, 4H=4096, F=512, NC=7):
    ctx   = v @ Wc.T + (bc + bi + bh)                      # [B, 4H], const over t
    per t: gates = x_t @ Wi.T + ctx + h @ Wh.T ; LSTM cell update
    out   = relu(h_T @ Wfa.T + bfa) @ Wfc.T + bfc          # [B, 7]

Strategy: pure data-parallel over batch, 32 rows per core, no collectives.

Key idea vs the v1 kernel: all three gate contributions accumulate in ONE
PSUM tile pair per step, and every matmul is column-tiled
(tile_position=(0,32j)) so four independent M=32 chunk-matmuls run
concurrently in the 128x128 PE array (~4x the PE efficiency of M=32 alone).
The x-projection for step t+LOOKAHEAD is emitted between the recurrence
matmuls of consecutive steps, so the PE never idles waiting for the cell-math
tail; there is no gx DRAM round trip at all.

PSUM layout per step: tile A rows = [i0|i1|f0|f1] (32 rows each), tile B =
[g0|g1|o0|o1], where e.g. i0 = i-gate columns 0:512. Activations then run as
[128,512]/[64,512] ops, and the cell products use same-base SBUF pairs plus
cross-quadrant DVE writes (verified legal+correct on HW by probe).
"""

import numpy as np
from contextlib import ExitStack

import concourse.bass as bass
import concourse.bacc as bacc
import concourse.mybir as mybir
import concourse.tile as tile
from concourse.bass_utils import run_bass_kernel_spmd

B, T, I = 256, 256, 1024
H = 1024
G = 4 * H
F = 512
NCLS = 7
NCORES = 8
BS = B // NCORES          # 32 batch rows per core
P = 128                   # partitions
KI = I // P               # 8 k-tiles over the input dim
KH = H // P               # 8 k-tiles over the hidden dim
TQ = 8                    # timesteps per x-prefetch DMA
LOOK = 2                  # steps of x-projection lookahead held in PSUM
F16 = mybir.dt.float16
F32 = mybir.dt.float32
AOP = mybir.AluOpType
AFT = mybir.ActivationFunctionType


def build_lstm(t_steps: int = T, rec_steps: int | None = None, reps: int = 1):
    assert rec_steps is None or rec_steps == t_steps
    assert t_steps % TQ == 0
    nc = bacc.Bacc("TRN2", target_bir_lowering=False, debug=False,
                   num_devices=NCORES)

    xT8 = nc.declare_dram_parameter("xT8", [t_steps // TQ, I, TQ, BS], F16,
                                    isOutput=False)
    vT = nc.declare_dram_parameter("vT", [I, BS], F16, isOutput=False)
    WiT = nc.declare_dram_parameter("WiT", [I, G], F16, isOutput=False)
    WhT = nc.declare_dram_parameter("WhT", [H, G], F16, isOutput=False)
    WcT = nc.declare_dram_parameter("WcT", [I, G], F16, isOutput=False)
    WfaT = nc.declare_dram_parameter("WfaT", [H, F], F16, isOutput=False)
    WfcT = nc.declare_dram_parameter("WfcT", [F, NCLS], F16, isOutput=False)
    bias = nc.declare_dram_parameter("bias_total", [G], F32, isOutput=False)
    bfa = nc.declare_dram_parameter("bfa", [F], F32, isOutput=False)
    bfc = nc.declare_dram_parameter("bfc", [NCLS], F32, isOutput=False)
    ident = nc.declare_dram_parameter("ident32", [BS, BS], F16, isOutput=False)
    out = nc.declare_dram_parameter("out", [BS, NCLS], F32, isOutput=True)

    # K-major views of DRAM tensors: i = k*128 + p
    xT8_r = xT8[:].rearrange("q (k p) s b -> q p k s b", p=P)
    WiT_r = WiT[:].rearrange("(k p) g -> p k g", p=P)
    WhT_r = WhT[:].rearrange("(k p) g -> p k g", p=P)
    WcT_r = WcT[:].rearrange("(k p) g -> p k g", p=P)
    WfaT_r = WfaT[:].rearrange("(k p) f -> p k f", p=P)
    WfcT_r = WfcT[:].rearrange("(q p) n -> p q n", p=P)
    vT_r = vT[:].rearrange("(k p) b -> p k b", p=P)

    def bcast(src_ap, rows):
        # read a [cols] DRAM vector into [rows, cols] SBUF (partition bcast)
        return bass.AP(tensor=src_ap.tensor, offset=src_ap.offset,
                       ap=[[0, rows]] + list(src_ap.ap))

    with tile.TileContext(nc) as tc, ExitStack() as ctx:
        consts = ctx.enter_context(tc.tile_pool(name="consts", bufs=1))

        # ---- resident constants ----
        bfa_rep = consts.tile([BS, F], F32, tag="bfa_rep")
        nc.sync.dma_start(out=bfa_rep, in_=bcast(bfa[:], BS))
        bfc_rep = consts.tile([BS, NCLS], F32, tag="bfc_rep")
        nc.sync.dma_start(out=bfc_rep, in_=bcast(bfc[:], BS))
        ident_sb = consts.tile([BS, BS], F16, tag="ident_sb")
        nc.sync.dma_start(out=ident_sb, in_=ident[:])

        WiT_sb = consts.tile([P, KI, G], F16, tag="WiT_sb")
        for k in range(KI):
            nc.sync.dma_start(out=WiT_sb[:, k, :], in_=WiT_r[:, k, :])
        WhT_sb = consts.tile([P, KH, G], F16, tag="WhT_sb")
        for k in range(KH):
            nc.sync.dma_start(out=WhT_sb[:, k, :], in_=WhT_r[:, k, :])
        WfaT_sb = consts.tile([P, KH, F], F16, tag="WfaT_sb")
        nc.sync.dma_start(out=WfaT_sb, in_=WfaT_r)
        WfcT_sb = consts.tile([P, F // P, NCLS], F16, tag="WfcT_sb")
        nc.sync.dma_start(out=WfcT_sb, in_=WfcT_r)

        # ctx chunks: ctxc[c] = (v @ Wc.T + bias_total)[:, 512c:512c+512], fp16
        ctxc = [consts.tile([BS, 512], F16, tag=f"ctx{c}", name=f"ctx{c}")
                for c in range(8)]

        for _rep in range(reps):
            # ------- phase 0: ctx -------
            with nc.named_scope("phase0_ctx"):
                with (
                    tc.tile_pool(name="ph0_w", bufs=2) as ph0_w,
                    tc.tile_pool(name="ph0_misc", bufs=1) as ph0_misc,
                    tc.tile_pool(name="ph0_ps", bufs=2, space="PSUM") as ph0_ps,
                ):
                    v_sb = ph0_misc.tile([P, KI, BS], F16, tag="v_sb")
                    nc.sync.dma_start(out=v_sb, in_=vT_r)
                    bias_rep = ph0_misc.tile([BS, G], F32, tag="bias_rep")
                    nc.sync.dma_start(out=bias_rep, in_=bcast(bias[:], BS))
                    for c in range(8):
                        wc_t = ph0_w.tile([P, KI, 512], F16, tag="wc")
                        nc.sync.dma_start(
                            out=wc_t, in_=WcT_r[:, :, 512 * c:512 * (c + 1)])
                        ps0 = ph0_ps.tile([BS, 512], F32, tag="ps0")
                        for k in range(KI):
                            nc.tensor.matmul(ps0, lhsT=v_sb[:, k, :],
                                             rhs=wc_t[:, k, :],
                                             start=(k == 0),
                                             stop=(k == KI - 1))
                        nc.vector.scalar_tensor_tensor(
                            out=ctxc[c], in0=ps0, scalar=1.0,
                            in1=bias_rep[:, 512 * c:512 * (c + 1)],
                            op0=AOP.mult, op1=AOP.add)

            # ------- fused main loop -------
            with (
                tc.tile_pool(name="xq", bufs=3) as xqp,
                tc.tile_pool(name="state", bufs=1) as statep,
                tc.tile_pool(name="acts", bufs=3) as actp,
                tc.tile_pool(name="cell", bufs=3) as cellp,
                tc.tile_pool(name="hh", bufs=3) as hhp,
                tc.tile_pool(name="ht", bufs=4) as htp,
            ):
                c_st = statep.tile([P, 512], F16, tag="c_st")  # rows 64:128 used
                nc.vector.memset(c_st[64:128, :], 0.0)

                xq_tiles = {}

                def fetch_xq(q):
                    if q * TQ >= t_steps:
                        return
                    xt = xqp.tile([P, KI, TQ, BS], F16, tag="xq")
                    nc.gpsimd.dma_start(out=xt, in_=xT8_r[q])
                    xq_tiles[q] = xt

                ps_tiles = {}

                def x_part(t, close):
                    # gates(t) += ctx + x_t @ Wi.T  (start of PSUM accumulation)
                    psA = psp.tile([P, 512], F32, tag="psA", name="psA")
                    psB = psp.tile([P, 512], F32, tag="psB", name="psB")
                    ps_tiles[t] = (psA, psB)
                    xt = xq_tiles[t // TQ][:, :, t % TQ, :]
                    for ti, ps in ((0, psA), (1, psB)):
                        for j in range(4):
                            nc.tensor.matmul(
                                ps[32 * j:32 * (j + 1), :], lhsT=ident_sb,
                                rhs=ctxc[4 * ti + j], start=True, stop=False,
                                tile_position=(0, 32 * j))
                    for k in range(KI):
                        for ti, ps in ((0, psA), (1, psB)):
                            for j in range(4):
                                c = 4 * ti + j
                                nc.tensor.matmul(
                                    ps[32 * j:32 * (j + 1), :],
                                    lhsT=xt[:, k, :],
                                    rhs=WiT_sb[:, k, 512 * c:512 * (c + 1)],
                                    start=False,
                                    stop=(close and k == KI - 1),
                                    tile_position=(0, 32 * j))

                def h_part(t, hT):
                    # tile A completes first so sigmoid(i,f) and t1/t2 prep
                    # overlap tile B's matmuls; the post-matmul chain is then
                    # only tanh(g)/sigmoid(o) -> t2 -> c' -> tanh -> h.
                    psA, psB = ps_tiles[t]
                    for ti, ps in ((0, psA), (1, psB)):
                        for k in range(KH):
                            for j in range(4):
                                c = 4 * ti + j
                                nc.tensor.matmul(
                                    ps[32 * j:32 * (j + 1), :],
                                    lhsT=hT[k],
                                    rhs=WhT_sb[:, k, 512 * c:512 * (c + 1)],
                                    start=False, stop=(k == KH - 1),
                                    tile_position=(0, 32 * j))

                def cell(t):
                    # psA = [i0|i1|f0|f1], psB = [g0|g1|o0|o1]; tile B's gates
                    # are ready ~1.8us before tile A's, so tanh(g)/sigmoid(o)
                    # run under tile A's matmuls.
                    psA, psB = ps_tiles.pop(t)
                    sA = actp.tile([P, 512], F16, tag="sA", name="sA")
                    nc.scalar.activation(out=sA, in_=psA, func=AFT.Sigmoid)
                    # t1 = sigmoid(f) * c  (only needs psA: runs under tile B)
                    t1t = cellp.tile([P, 512], F16, tag="t1t", name="t1t")
                    nc.vector.tensor_tensor(out=t1t[64:128, :],
                                            in0=sA[64:128, :],
                                            in1=c_st[64:128, :], op=AOP.mult)
                    gb = actp.tile([P, 512], F16, tag="gb", name="gb")
                    nc.scalar.activation(out=gb[0:64, :], in_=psB[0:64, :],
                                         func=AFT.Tanh)
                    nc.scalar.activation(out=gb[64:128, :], in_=psB[64:128, :],
                                         func=AFT.Sigmoid)
                    # t2 = sigmoid(i) * tanh(g)  -> rows 64:128  (fp16 chain)
                    t2t = cellp.tile([P, 512], F16, tag="t2t", name="t2t")
                    nc.vector.tensor_tensor(out=t2t[64:128, :],
                                            in0=sA[0:64, :], in1=gb[0:64, :],
                                            op=AOP.mult)
                    nc.vector.tensor_tensor(out=c_st[64:128, :],
                                            in0=t1t[64:128, :],
                                            in1=t2t[64:128, :], op=AOP.add)
                    tct = cellp.tile([P, 512], F16, tag="tct", name="tct")
                    nc.scalar.activation(out=tct[64:128, :],
                                         in_=c_st[64:128, :], func=AFT.Tanh)
                    # h = sigmoid(o) * tanh(c') -> rows 0:64, fp16
                    hh = hhp.tile([P, 512], F16, tag="hh", name="hh")
                    nc.vector.tensor_tensor(out=hh[0:64, :],
                                            in0=gb[64:128, :],
                                            in1=tct[64:128, :], op=AOP.mult)
                    # one xbar transpose: [64,512] -> [128, 4, 64];
                    # hT[k] = h[:, 128k:128(k+1)].T at [:, k%4, 32*(k//4):...]
                    htn = htp.tile([P, 4, 2 * BS], F16, tag="ht", name="htn")
                    nc.sync.dma_start_transpose(out=htn, in_=hh[0:64, :])
                    return [htn[:, q, 0:BS] for q in range(4)] + \
                           [htn[:, q, BS:2 * BS] for q in range(4)]

                # prologue: x prefetch + LOOK steps of x-projection
                with tc.tile_pool(name="ps", bufs=4, space="PSUM") as psp:
                    fetch_xq(0)
                    fetch_xq(1)
                    for t in range(LOOK):
                        x_part(t, close=(t == 0))

                    hT = None
                    for t in range(t_steps):
                        tp = t + LOOK
                        if tp < t_steps:
                            if tp % TQ == 0:
                                fetch_xq(tp // TQ + 1)
                            x_part(tp, close=False)
                        if t > 0:
                            h_part(t, hT)
                        hT = cell(t)

                # ------- head -------
                with nc.named_scope("head"):
                    with tc.tile_pool(name="head_ps", bufs=1,
                                      space="PSUM") as hps:
                        ps_f = hps.tile([BS, F], F32, tag="ps_f")
                        for k in range(KH):
                            nc.tensor.matmul(ps_f, lhsT=hT[k],
                                             rhs=WfaT_sb[:, k, :],
                                             start=(k == 0),
                                             stop=(k == KH - 1))
                        x1 = cellp.tile([BS, F], F32, tag="x1", bufs=1)
                        nc.vector.scalar_tensor_tensor(
                            out=x1, in0=ps_f, scalar=1.0, in1=bfa_rep,
                            op0=AOP.mult, op1=AOP.add)
                        x1r = cellp.tile([BS, F], F16, tag="x1r", bufs=1)
                        nc.scalar.activation(out=x1r, in_=x1, func=AFT.Relu)
                        x1T = htp.tile([P, F // P, BS], F16, tag="x1T",
                                       bufs=1)
                        nc.sync.dma_start_transpose(out=x1T, in_=x1r)
                        ps_o = hps.tile([BS, NCLS], F32, tag="ps_o")
                        for q in range(F // P):
                            nc.tensor.matmul(ps_o, lhsT=x1T[:, q, :],
                                             rhs=WfcT_sb[:, q, :],
                                             start=(q == 0),
                                             stop=(q == F // P - 1))
                        out_sb = cellp.tile([BS, NCLS], F32, tag="out_sb",
                                            bufs=1)
                        nc.vector.scalar_tensor_tensor(
                            out=out_sb, in0=ps_o, scalar=1.0, in1=bfc_rep,
                            op0=AOP.mult, op1=AOP.add)
                        nc.sync.dma_start(out=out[:], in_=out_sb)

    nc.compile()
    return nc


def make_in_maps(inputs: dict, t_steps: int = T):
    """Shard + lay out the full inputs for the 8 cores (host-side numpy)."""
    x = np.asarray(inputs["i_features"], np.float32)[:, :t_steps, :]
    v = np.asarray(inputs["v_features"], np.float32)
    Wi, bi = np.asarray(inputs["Wi"], np.float32), np.asarray(inputs["bi"], np.float32)
    Wh, bh = np.asarray(inputs["Wh"], np.float32), np.asarray(inputs["bh"], np.float32)
    Wc, bc = np.asarray(inputs["Wc"], np.float32), np.asarray(inputs["bc"], np.float32)
    Wfa, bfa = np.asarray(inputs["Wfa"], np.float32), np.asarray(inputs["bfa"], np.float32)
    Wfc, bfc = np.asarray(inputs["Wfc"], np.float32), np.asarray(inputs["bfc"], np.float32)

    shared = {
        "WiT": np.ascontiguousarray(Wi.T).astype(np.float16),
        "WhT": np.ascontiguousarray(Wh.T).astype(np.float16),
        "WcT": np.ascontiguousarray(Wc.T).astype(np.float16),
        "WfaT": np.ascontiguousarray(Wfa.T).astype(np.float16),
        "WfcT": np.ascontiguousarray(Wfc.T).astype(np.float16),
        "bias_total": (bi + bh + bc).astype(np.float32),
        "bfa": bfa.astype(np.float32),
        "bfc": bfc.astype(np.float32),
        "ident32": np.eye(BS, dtype=np.float16),
    }
    in_maps = []
    nb = x.shape[0] // BS
    for s in range(nb):
        xs = x[s * BS:(s + 1) * BS]                      # [BS, t, I]
        xT = np.ascontiguousarray(xs.transpose(1, 2, 0))  # [t, I, BS]
        xT8 = np.ascontiguousarray(
            xT.reshape(t_steps // TQ, TQ, I, BS).transpose(0, 2, 1, 3))
        in_maps.append({
            "xT8": xT8.astype(np.float16),
            "vT": np.ascontiguousarray(v[s * BS:(s + 1) * BS].T).astype(np.float16),
            **shared,
        })
    return in_maps


_NC_CACHE = {}


def finish_output(per_core_outs: list) -> np.ndarray:
    """Host-side gather of per-core 'out' tensors into the full [B, NC] result."""
    return np.concatenate(per_core_outs, axis=0).astype(np.float32)


def kernel(**inputs) -> np.ndarray:
    in_maps = make_in_maps(inputs, T)
    if T not in _NC_CACHE:
        _NC_CACHE[T] = build_lstm(T)
    nc = _NC_CACHE[T]
    res = run_bass_kernel_spmd(nc, in_maps, core_ids=list(range(NCORES)))
    return finish_output([r["out"] for r in res.results])
